# revision 5
# baseline (speedup 1.0000x reference)
"""ARIMA(0,1,1) innovations kernel for 8 TRN2 NeuronCores.

Math: the reference solves the min-norm least-squares problem A x = b where
A is the N x (N+1) bidiagonal MA(1) matrix (c on the diagonal, 1 on the
superdiagonal), b = diff(time_block) - arma_const, and returns x / std.

Every x with A x = b satisfies x_{i+1} = b_i - c*x_i, so the solution set is
x(t) = xhat + t*h with xhat = [0, f] (f the IIR scan f_i = s*f_{i-1} + b_i,
s = -c, i.e. A xhat = b with xhat_0 = 0) and h_i = s^i spanning null(A).
The min-norm solution projects out h:  x = xhat - (<xhat,h>/||h||^2) h.
With |c| < 1,  rho := <xhat,h>/||h||^2 = sum_j b_j s^{j+1}  exactly (to f32;
the dropped terms are O(s^N), N = 4096).  So the whole problem is one
first-order scan plus one rank-one correction.

On-chip layout: b is blocked [128, 32] (partition p holds elements
32p..32p+31).  Local scans run as a single native tensor_tensor_scan; the
cross-partition carries C[p] = s^32*C[p-1] + f_loc[p,31] are themselves an
exact 128-length scan computed in a transposed [1,128] row (transpose via
PE matmul against identity), then transposed back into per-partition
initial states for a second scan pass.  The rank-one correction
-rho*s^{32p+q+1} is folded into that initial state (it obeys the same
recurrence), as a second matmul accumulating -rho*s^{32p} into the same
PSUM bank.  All s-power vectors are generated exactly on device by the
same scan instruction.  Scalar params are broadcast across partitions with
K=1 matmuls.  No collectives: the problem is tiny (16 KB in/out), so all
8 cores run the identical program (data-parallel replication per the
sharding hint) and the host takes core 0's output.

Assumes 0 < |ma_coeff| < 1 (reference setup uses c = 0.5; at |c| -> 1 the
geometric-series identity for the projection coefficient degrades).
"""

import numpy as np

N = 4096
P = 128
Q = 32

_CACHE: dict = {}


def _ensure_paths():
    import sys
    for p in ("/opt/trn_rl_repo", "/root/.axon_site", "/root/.axon_site/_ro/trn_rl_repo",
              "/root/.axon_site/_ro/pypackages"):
        if p not in sys.path:
            sys.path.append(p)


def build_nc():
    """Build the Bass/Tile graph (single SPMD program, run on all 8 cores)."""
    _ensure_paths()
    import concourse.bass as bass
    import concourse.mybir as mybir
    from concourse import bacc, tile

    dt = mybir.dt.float32
    OP = mybir.AluOpType

    nc = bacc.Bacc(None, target_bir_lowering=False)

    tb_d = nc.dram_tensor("time_block", [N + 1], dt, kind="ExternalInput")
    const_d = nc.dram_tensor("arma_const", [1], dt, kind="ExternalInput")
    coeff_d = nc.dram_tensor("ma_coeff", [1], dt, kind="ExternalInput")
    std_d = nc.dram_tensor("std_innovation", [1], dt, kind="ExternalInput")
    ident_d = nc.dram_tensor("ident", [P, P], dt, kind="ExternalInput")
    out_d = nc.dram_tensor("out", [N + 1], dt, kind="ExternalOutput")

    with tile.TileContext(nc) as tc:
        with (
            tc.tile_pool(name="sb", bufs=1) as sb,
            tc.tile_pool(name="ps", bufs=1, space=bass.MemorySpace.PSUM) as ps,
        ):
            TBa = sb.tile([P, Q], dt)
            TBb = sb.tile([P, Q], dt)
            IDT = sb.tile([P, P], dt)
            c1t = sb.tile([1, 1], dt)
            cct = sb.tile([1, 1], dt)
            sdt = sb.tile([1, 1], dt)
            SC = sb.tile([1, 16], dt)      # scalar scratch row (partition 0)
            ONESR = sb.tile([1, P], dt)    # ones row
            ZROW = sb.tile([1, P], dt)     # zero row
            BCT = sb.tile([P, 3], dt)      # [const, s, istd] broadcast
            ZT = sb.tile([P, Q], dt)       # zeros
            ST = sb.tile([P, Q], dt)       # s replicated
            POW = sb.tile([P, Q + 1], dt)  # POW[p, j] = s^j
            S32R = sb.tile([1, P], dt)     # s^32 replicated row
            SPROW = sb.tile([1, P + 1], dt)  # col j = s^{32j}
            B0 = sb.tile([P, Q], dt)
            B = sb.tile([P, Q], dt)
            FLOC = sb.tile([P, Q], dt)
            CR = sb.tile([1, P + 1], dt)   # carry row, col p+1 = C[p]
            Ff = sb.tile([P, Q], dt)
            F = sb.tile([P, Q], dt)
            WP = sb.tile([P, Q], dt)       # B * s^{q+1}
            WC = sb.tile([P, 1], dt)       # row sums of WP
            SPC = sb.tile([P, 1], dt)      # s^{32p} column
            RHOc = sb.tile([1, 1], dt)     # rho
            NEGR = sb.tile([1, 1], dt)     # -rho
            O0 = sb.tile([1, 1], dt)       # -rho*istd

            psBC = ps.tile([P, 3], dt)
            psROW = ps.tile([1, P], dt)
            psC = ps.tile([P, 1], dt)
            psS = ps.tile([P, 1], dt)
            psR = ps.tile([1, 1], dt)

            V = nc.vector
            G = nc.gpsimd
            tt = V.tensor_tensor
            tts = V.tensor_tensor_scan
            gt = G.tensor_tensor

            # ---- input DMAs, spread across the three DMA-capable queues;
            # scalars first (they gate the setup chain) ----
            nc.sync.dma_start(out=cct[:], in_=coeff_d[None, :])
            nc.scalar.dma_start(out=c1t[:], in_=const_d[None, :])
            G.dma_start(out=sdt[:], in_=std_d[None, :])
            nc.sync.dma_start(out=TBa[:], in_=tb_d[0:N].rearrange("(p q) -> p q", p=P))
            nc.scalar.dma_start(out=TBb[:], in_=tb_d[1:N + 1].rearrange("(p q) -> p q", p=P))
            G.dma_start(out=IDT[:], in_=ident_d[:])

            # ---- compile-time constants (gpsimd) ----
            G.memset(ONESR[:], 1.0)
            G.memset(ZROW[:], 0.0)
            G.memset(ZT[:], 0.0)
            G.memset(CR[0:1, 0:1], 0.0)
            G.memset(SPROW[0:1, 0:1], 1.0)
            G.memset(POW[:, 0:1], 1.0)

            # ---- scalar assembly on partition 0 (DVE) ----
            # SC cols: 0=const 1=s 2=istd 3=s2 4=s4 5=s8 6=s16 8=s32
            V.tensor_copy(SC[0:1, 0:1], c1t[:])
            V.tensor_scalar_mul(SC[0:1, 1:2], cct[:], -1.0)
            V.reciprocal(SC[0:1, 2:3], sdt[:])
            tt(SC[0:1, 3:4], SC[0:1, 1:2], SC[0:1, 1:2], OP.mult)
            tt(SC[0:1, 4:5], SC[0:1, 3:4], SC[0:1, 3:4], OP.mult)
            tt(SC[0:1, 5:6], SC[0:1, 4:5], SC[0:1, 4:5], OP.mult)
            tt(SC[0:1, 6:7], SC[0:1, 5:6], SC[0:1, 5:6], OP.mult)
            tt(SC[0:1, 8:9], SC[0:1, 6:7], SC[0:1, 6:7], OP.mult)

            # broadcast [const, s, istd] to all partitions
            nc.tensor.matmul(psBC[:], ONESR[0:1, 0:P], SC[0:1, 0:3])
            V.tensor_copy(BCT[:], psBC[:])
            constB = BCT[:, 0:1]
            sB = BCT[:, 1:2]
            istdB = BCT[:, 2:3]

            # s tile + power vectors (exact scans on DVE; fill on gpsimd)
            V.tensor_scalar_add(ST[:], ZT[:], sB)
            G.tensor_scalar_mul(S32R[:], ONESR[:], SC[0:1, 8:9])
            tts(POW[:, 1:Q + 1], ST[:], ZT[:], 1.0, OP.mult, OP.add)
            tts(SPROW[0:1, 1:P + 1], S32R[:], ZROW[:], 1.0, OP.mult, OP.add)
            nc.tensor.matmul(psS[:], SPROW[0:1, 0:P], ONESR[0:1, 0:1])
            V.tensor_copy(SPC[:], psS[:, 0:1])

            # ---- main chain (DVE + PE) ----
            tt(B0[:], TBb[:], TBa[:], OP.subtract)
            V.tensor_scalar_sub(B[:], B0[:], constB)
            tts(FLOC[:], ST[:], B[:], 0.0, OP.mult, OP.add)
            nc.tensor.matmul(psROW[:], FLOC[:, Q - 1:Q], IDT[:])
            tts(CR[0:1, 1:P + 1], S32R[:], psROW[:], 0.0, OP.mult, OP.add)

            # ---- rho chain (gpsimd + PE), racing the carry chain ----
            gt(WP[:], B[:], POW[:, 1:Q + 1], OP.mult)
            V.tensor_reduce(WC[:], WP[:], mybir.AxisListType.X, OP.add)
            nc.tensor.matmul(psR[:], WC[:, 0:1], SPC[:, 0:1])
            V.tensor_copy(RHOc[:], psR[:])
            G.tensor_scalar_mul(NEGR[:], RHOc[:], -1.0)
            gt(O0[:], NEGR[:], SC[0:1, 2:3], OP.mult)  # x_0 = -rho*istd

            # carry column + folded correction: psC[p] = C[p-1] - rho*s^{32p}
            nc.tensor.matmul(psC[:], CR[0:1, 0:P], ONESR[0:1, 0:1],
                             start=True, stop=False)
            nc.tensor.matmul(psC[:], SPROW[0:1, 0:P], NEGR[0:1, 0:1],
                             start=False, stop=True)

            # final scan directly yields f - rho*s^{32p+q+1}
            tts(Ff[:], ST[:], B[:], psC[:, 0:1], OP.mult, OP.add)
            V.tensor_scalar_mul(F[:], Ff[:], istdB)

            # ---- stores ----
            nc.sync.dma_start(out=out_d[1:N + 1].rearrange("(p q) -> p q", p=P), in_=F[:])
            nc.scalar.dma_start(out=out_d[0:1][None, :], in_=O0[:])

    nc.compile()
    return nc


def _get_nc():
    if "nc" not in _CACHE:
        _CACHE["nc"] = build_nc()
    return _CACHE["nc"]


def _in_map(inputs):
    return {
        "time_block": np.ascontiguousarray(np.asarray(inputs["time_block"], dtype=np.float32)),
        "arma_const": np.ascontiguousarray(np.asarray(inputs["arma_const"], dtype=np.float32)),
        "ma_coeff": np.ascontiguousarray(np.asarray(inputs["ma_coeff"], dtype=np.float32)),
        "std_innovation": np.ascontiguousarray(np.asarray(inputs["std_innovation"], dtype=np.float32)),
        "ident": np.eye(P, dtype=np.float32),
    }


def run(inputs, trace=False, tmpdir=None):
    """Run on all 8 cores (replicated); returns (output, BassKernelResults)."""
    _ensure_paths()
    from concourse.bass_utils import run_bass_kernel_spmd

    nc = _get_nc()
    m = _in_map(inputs)
    res = run_bass_kernel_spmd(nc, [m] * 8, list(range(8)), trace=trace, tmpdir=tmpdir)
    return res.results[0]["out"].reshape(N + 1).astype(np.float32), res


def kernel(**inputs) -> np.ndarray:
    out, _ = run(inputs)
    return out
